# revision 27
# baseline (speedup 1.0000x reference)
"""Causal self-attention (B=2, T=4096, C=768, H=12, D=64) on 8 trn2 cores.

Sharding: (B, H) -> 24 (batch, head) pairs, 3 heads per core.
Core c handles batch b = c // 4 and heads 3*(c%4) .. 3*(c%4)+2.

v5: fine-grained weave + trimmed diagonal. Engines run their queues in
order, so overlap is encoded in emission order: qkv projection units
for block j+1 AND the renorm/output-projection of block j-1 are woven
between the attention pairs of block j, keeping the scalar engine's
exp stream (the per-core floor, ~230us) continuous. Score matmuls are
fully row-paired: h0/h1 across partition halves, h2 against itself
(duplicated q/k in partitions 64-127). The 4 diagonal key-tiles of
each block use trimmed query ranges (512/384/256/128) so ~11% of
score/exp/PV work on masked-out regions is skipped; only the leading
128-query triangle of each tile needs a mask. The output projection
contracts h0+h1 in one K=128 matmul (heads stacked on partitions).

Per-core layouts (all bf16 unless noted):
  xT   [768, 4096]   x[b].T so projections stream tokens in the free dim
  wqk  [768, 3, 128] col groups: [Wq_h0|Wq_h1], [Wk_h0|Wk_h1], [Wq_h2|Wk_h2]
  wvp  [768, 192]    [Wv_h0 Wv_h1 Wv_h2]
  wp01 [128, 768]    Wproj rows for h0 (partitions 0-63) and h1 (64-127)
  wp2  [64, 768]     Wproj rows for h2
  consts [128, 2048] 4 causal masks [128,512]
Scores are computed transposed (ST[k, q]) so the PV matmul contracts k
on the partition dim with V in natural [t, d] layout. Row sums ride an
appended ones-column on V (flash-style, no running max).
"""

import numpy as np

B, T, C, H, D = 2, 4096, 768, 12, 64
HPC = 3          # heads per core
NCORES = 8
QB = 512         # query block (psum bank width in fp32)
KT = 128         # key tile
NKT = T // KT    # 32
NQB = T // QB    # 8
VSTRIDE = 264    # per-k-tile column stride in vbig (3*65 used + pad so a
                 # [128,128] PV stationary slice fits at offset 130)

_COMPILED = {}
LAST = {}


def _emit(nc, tile, mybir, tc, ctx, aps):
    F32 = mybir.dt.float32
    BF16 = mybir.dt.bfloat16
    EXP = mybir.ActivationFunctionType.Exp
    xT, wqk, wvp, wp01, wp2, consts, out = aps
    CC = C // 128  # 6 contraction chunks for the projections

    wpool = ctx.enter_context(tc.tile_pool(name="w", bufs=1))
    qkvpool = ctx.enter_context(tc.tile_pool(name="qkv", bufs=1))
    xpool = ctx.enter_context(tc.tile_pool(name="x", bufs=6))
    ptpool = ctx.enter_context(tc.tile_pool(name="pt", bufs=9))
    atpool = ctx.enter_context(tc.tile_pool(name="at", bufs=2))
    opool = ctx.enter_context(tc.tile_pool(name="osb", bufs=3))
    rpool = ctx.enter_context(tc.tile_pool(name="r", bufs=3))
    stp = ctx.enter_context(tc.tile_pool(name="stp", bufs=2, space="PSUM"))
    osp = ctx.enter_context(tc.tile_pool(name="osp", bufs=3, space="PSUM"))
    pjp = ctx.enter_context(tc.tile_pool(name="pjp", bufs=1, space="PSUM"))

    # ---- constants and weights (wqk first: first matmul needs only it + x) ----
    wqk_sb = wpool.tile([128, CC * 3 * 128], BF16)
    nc.sync.dma_start(
        wqk_sb[:].rearrange("p (a g m) -> p a g m", a=CC, g=3),
        wqk.rearrange("(a p) g m -> p a g m", p=128),
    )
    xpre = []
    for tb in range(2):
        for half in range(2):
            xt = xpool.tile([128, 3 * QB], BF16, tag="xt", name=f"xpre{tb}_{half}")
            nc.sync.dma_start(
                xt[:].rearrange("p (a t) -> p a t", a=3),
                xT[
                    384 * half : 384 * (half + 1), tb * QB : (tb + 1) * QB
                ].rearrange("(a p) t -> p a t", p=128),
            )
            xpre.append(xt)
    wvp_sb = wpool.tile([128, CC * 192], BF16)
    nc.sync.dma_start(
        wvp_sb[:].rearrange("p (a n) -> p a n", a=CC),
        wvp.rearrange("(a p) n -> p a n", p=128),
    )
    # tril mask [128,128] stored twice side by side (strided double-mask
    # for the two 128-query triangle blocks of each trimmed diagonal tile)
    masks_sb = wpool.tile([128, 256], BF16)
    nc.sync.dma_start(masks_sb[:], consts[:])
    wp01_sb = wpool.tile([128, C], BF16)
    nc.sync.dma_start(wp01_sb[:], wp01)
    wp2_sb = wpool.tile([64, C], BF16)
    nc.sync.dma_start(wp2_sb[:], wp2)

    # ---- qkv storage ----
    # qkT01: [0:T] = qT (h0 rows 0-63, h1 rows 64-127), [T:2T] = kT
    qkT01 = qkvpool.tile([128, 2 * T], BF16)
    # qk2: h2's qT/kT duplicated in both partition halves so its two
    # key-tile score matmuls of a pair can run row-concurrent.
    qk2 = qkvpool.tile([128, 2 * T], BF16)
    vbig = qkvpool.tile([128, NKT * VSTRIDE], BF16)
    vbig3 = vbig[:].rearrange("p (t c) -> p t c", c=VSTRIDE)
    for h in range(3):
        nc.gpsimd.memset(vbig3[:, :, 65 * h + 64 : 65 * h + 65], 1.0)
    # zero the pad tail so h2's [128,128] PV stationary never reads
    # uninitialized SBUF (NaN bits would poison the junk partitions)
    nc.gpsimd.memset(vbig3[:, :, 195:VSTRIDE], 0.0)

    def emit_qkv_units(tb):
        """Generator: yields after each ~1-1.5us PE unit."""
        t0 = tb * QB
        if tb < 2:
            xh = xpre[2 * tb : 2 * tb + 2]
        else:
            xh = []
            for half in range(2):
                xt = xpool.tile([128, 3 * QB], BF16, tag="xt")
                nc.sync.dma_start(
                    xt[:].rearrange("p (a t) -> p a t", a=3),
                    xT[384 * half : 384 * (half + 1), t0 : t0 + QB].rearrange(
                        "(a p) t -> p a t", p=128
                    ),
                )
                xh.append(xt)

        def xchunk(cc):
            return xh[cc // 3][:, (cc % 3) * QB : (cc % 3 + 1) * QB]

        # unit: q01 and k01 projections (one [128,1024] psum tile)
        ps_qk = stp.tile([128, 2 * QB], F32, tag="st")
        for cc in range(CC):
            nc.tensor.matmul(
                ps_qk[:, 0:QB],
                wqk_sb[:, (cc * 3 + 0) * 128 : (cc * 3 + 1) * 128],
                xchunk(cc),
                start=(cc == 0),
                stop=(cc == CC - 1),
            )
        nc.vector.tensor_copy(qkT01[:, t0 : t0 + QB], ps_qk[:, 0:QB])
        yield
        for cc in range(CC):
            nc.tensor.matmul(
                ps_qk[:, QB : 2 * QB],
                wqk_sb[:, (cc * 3 + 1) * 128 : (cc * 3 + 2) * 128],
                xchunk(cc),
                start=(cc == 0),
                stop=(cc == CC - 1),
            )
        nc.vector.tensor_copy(qkT01[:, T + t0 : T + t0 + QB], ps_qk[:, QB : 2 * QB])
        yield
        # unit: h2 q (psum rows 0-63, bank 0) and k (rows 64-127, bank 1)
        # col-paired; distinct 2KB regions so start-zeroing can't collide.
        ps_h2 = stp.tile([128, 2 * QB], F32, tag="st")
        for g2 in range(2):
            for cc in range(CC):
                base = (cc * 3 + 2) * 128 + 64 * g2
                nc.tensor.matmul(
                    ps_h2[64 * g2 : 64 * g2 + 64, g2 * QB : (g2 + 1) * QB],
                    wqk_sb[:, base : base + 64],
                    xchunk(cc),
                    start=(cc == 0),
                    stop=(cc == CC - 1),
                )
        nc.vector.tensor_copy(qk2[0:64, t0 : t0 + QB], ps_h2[0:64, 0:QB])
        nc.vector.tensor_copy(qk2[64:128, t0 : t0 + QB], ps_h2[0:64, 0:QB])
        nc.vector.tensor_copy(qk2[0:64, T + t0 : T + t0 + QB], ps_h2[64:128, QB : 2 * QB])
        nc.vector.tensor_copy(
            qk2[64:128, T + t0 : T + t0 + QB], ps_h2[64:128, QB : 2 * QB]
        )
        yield
        # units: v projection, one per k-tile
        for tt in range(4):
            kt = 4 * tb + tt
            ps = pjp.tile([128, QB], F32, tag="m")
            for cc in range(CC):
                nc.tensor.matmul(
                    ps[:, 0:192],
                    xchunk(cc)[:, tt * 128 : (tt + 1) * 128],
                    wvp_sb[:, cc * 192 : (cc + 1) * 192],
                    start=(cc == 0),
                    stop=(cc == CC - 1),
                )
            dst = vbig3[:, kt, 0:195].rearrange("p (h c) -> p h c", c=65)[:, :, 0:64]
            nc.vector.tensor_copy(dst, ps[:, 0:192].rearrange("p (h d) -> p h d", h=3))
            yield

    # score-matmul helper: one score MM for head h, key-tile kt, query
    # range [q0, q0+w), output at st[:, c0:c0+w]. For h0/h1 the row half
    # is fixed (cross-head pairing); for h2 it alternates per kt (self
    # pairing against the duplicated copy in partitions 64-127).
    def score_mm(st, h, kt, q0, w, c0, half):
        if h < 2:
            nc.tensor.matmul(
                st[:, c0 : c0 + w],
                qkT01[64 * h : 64 * h + 64, T + kt * KT : T + (kt + 1) * KT],
                qkT01[64 * h : 64 * h + 64, q0 : q0 + w],
                start=True,
                stop=True,
                tile_position=(64 * h, 0),
            )
        else:
            nc.tensor.matmul(
                st[:, c0 : c0 + w],
                qk2[64 * half : 64 * half + 64, T + kt * KT : T + (kt + 1) * KT],
                qk2[64 * half : 64 * half + 64, q0 : q0 + w],
                start=True,
                stop=True,
                tile_position=(64 * half, 0),
            )

    # PV stationary is a full [128,128] slice (64 v cols + ones col + 63
    # junk cols of the neighbor head) so all 4 PE column groups engage —
    # M=65 stationaries issue ~90ns slower. o_ps partitions 65-127 hold
    # junk accumulation and are never read.
    def pv_mm(o_ps, h, kt, nkt, pt, c0, w, qoff):
        nc.tensor.matmul(
            o_ps[h][:, qoff : qoff + w],
            vbig3[:, kt, 65 * h : 65 * h + 128],
            pt[:, c0 : c0 + w],
            start=(kt == 0),
            stop=(kt == nkt - 1),
        )

    # PV matmuls run one group (pair / diag half) behind their scores:
    # `pending` holds thunks for the previous group's PVs, flushed between
    # this group's h0/h1 ACTs and the h2 score matmuls. So every PV's pt
    # has been ready for ~3us (no exposed dependency latency), the PE has
    # dense work during the ACT latency, and the exp stream stays fed.
    pending = []

    def flush_pending():
        for fn in pending:
            fn()
        pending.clear()

    # off-diagonal pair: two full-width key tiles, no mask.
    # h0/h1 score matmuls interleaved so the row-group pairs issue
    # concurrently (3 slots per 2 key-tiles instead of 4).
    def emit_pair(o_ps, qb, g):
        t0 = qb * QB
        nkt = 4 * qb + 4
        sts = [
            stp.tile([128, 2 * QB], F32, tag="st", name=f"st{qb}_{g}_{h}")
            for h in range(2)
        ]
        for i in range(2):
            for h in range(2):
                score_mm(sts[h], h, 2 * g + i, t0, QB, i * QB, i)
        pts = []
        for h in range(2):
            pt = ptpool.tile([128, 2 * QB], BF16, tag="pt")
            nc.scalar.activation(pt[:], sts[h][:], EXP, scale=float(D) ** -0.5)
            pts.append(pt)
        flush_pending()
        st2 = stp.tile([128, 2 * QB], F32, tag="st", name=f"st{qb}_{g}_2")
        for i in range(2):
            score_mm(st2, 2, 2 * g + i, t0, QB, i * QB, i)
        pt2 = ptpool.tile([128, 2 * QB], BF16, tag="pt")
        nc.scalar.activation(pt2[:], st2[:], EXP, scale=float(D) ** -0.5)
        pts.append(pt2)

        def make_pv(h, i, pt):
            return lambda: pv_mm(o_ps, h, 2 * g + i, nkt, pt, i * QB, QB, 0)

        for i in range(2):
            for h in range(3):
                pending.append(make_pv(h, i, pts[h]))

    # diagonal block: 4 key tiles with trimmed query ranges.
    # tile A holds kt d0 (w=512 at col 0) + d1 (w=384 at col 512);
    # tile B holds kt d2 (w=256 at col 0) + d3 (w=128 at col 512).
    # Each start=True matmul owns its own 2KB PSUM region. The leading
    # 128 queries of each segment get the tril mask (strided double-mask
    # covers cols [0:128] and [512:640] in one DVE op).
    def emit_diag(o_ps, qb):
        t0 = qb * QB
        nkt = 4 * qb + 4
        segs = ((0, 512, 128, 384), (256, 256, 384, 128))  # (qoff0, w0, qoff1, w1)
        for pi, (qo0, w0, qo1, w1) in enumerate(segs):
            kd0, kd1 = 4 * qb + 2 * pi, 4 * qb + 2 * pi + 1

            def mask_pt(pt):
                ptm = pt[:].rearrange("p (a c) -> p a c", c=128)[:, 0:5:4, :]
                nc.vector.tensor_mul(
                    ptm, ptm, masks_sb[:].rearrange("p (a c) -> p a c", c=128)
                )

            sts = [
                stp.tile([128, 2 * QB], F32, tag="st", name=f"std{qb}_{pi}_{h}")
                for h in range(2)
            ]
            for h in range(2):
                score_mm(sts[h], h, kd0, t0 + qo0, w0, 0, 0)
            for h in range(2):
                score_mm(sts[h], h, kd1, t0 + qo1, w1, QB, 1)
            pts = []
            for h in range(2):
                pt = ptpool.tile([128, 2 * QB], BF16, tag="pt")
                nc.scalar.activation(
                    pt[:, 0 : QB + w1], sts[h][:, 0 : QB + w1], EXP,
                    scale=float(D) ** -0.5,
                )
                mask_pt(pt)
                pts.append(pt)
            flush_pending()
            st2 = stp.tile([128, 2 * QB], F32, tag="st", name=f"std{qb}_{pi}_2")
            score_mm(st2, 2, kd0, t0 + qo0, w0, 0, 0)
            score_mm(st2, 2, kd1, t0 + qo1, w1, QB, 1)
            pt2 = ptpool.tile([128, 2 * QB], BF16, tag="pt")
            nc.scalar.activation(
                pt2[:, 0 : QB + w1], st2[:, 0 : QB + w1], EXP, scale=float(D) ** -0.5
            )
            mask_pt(pt2)
            pts.append(pt2)

            def make_pv(h, kd, pt, c0, w, qo):
                return lambda: pv_mm(o_ps, h, kd, nkt, pt, c0, w, qo)

            for h in range(3):
                pending.append(make_pv(h, kd0, pts[h], 0, w0, qo0))
            for h in range(3):
                pending.append(make_pv(h, kd1, pts[h], QB, w1, qo1))

    def emit_renorm_for(qb, o_ps):
        att01 = atpool.tile([128, QB], BF16, tag="att01", name=f"att01_{qb}")
        att2 = atpool.tile([64, QB], BF16, tag="att2", name=f"att2_{qb}")
        for h in range(3):
            au = atpool.tile([64, QB], F32, tag=f"au{h}", name=f"au{qb}_{h}")
            nc.vector.tensor_copy(au[:], o_ps[h][0:64, :])
            srow = rpool.tile([1, QB], F32, tag="sr")
            nc.vector.tensor_copy(srow[:], o_ps[h][64:65, :])
            rs = rpool.tile([1, QB], F32, tag="r")
            nc.vector.reciprocal_approx_fast(rs[:], srow[:])
            rbc = atpool.tile([64, QB], F32, tag=f"rbc{h}", name=f"rbc{qb}_{h}")
            nc.gpsimd.partition_broadcast(rbc[:], rs[:])
            dstmul = (att01[0:64, :], att01[64:128, :], att2[:])[h]
            nc.vector.tensor_mul(dstmul, au[:], rbc[:])
        return (att01, att2)

    def emit_outproj_tt(qb, tt, att01, att2):
        t0 = qb * QB
        osb = opool.tile([128, C], F32, tag="osb")
        for j, (c0, cw) in enumerate(((0, 512), (512, 256))):
            pps = pjp.tile([128, QB], F32, tag="m")
            nc.tensor.matmul(
                pps[:, 0:cw],
                att01[:, tt * 128 : (tt + 1) * 128],
                wp01_sb[:, c0 : c0 + cw],
                start=True,
                stop=False,
            )
            nc.tensor.matmul(
                pps[:, 0:cw],
                att2[:, tt * 128 : (tt + 1) * 128],
                wp2_sb[:, c0 : c0 + cw],
                start=False,
                stop=True,
            )
            nc.vector.tensor_copy(osb[:, c0 : c0 + cw], pps[:, 0:cw])
        nc.sync.dma_start(out[t0 + tt * 128 : t0 + (tt + 1) * 128, :], osb[:])

    # weave: a single stream of qkv units (all token blocks in sequence)
    # plus a deferred-work queue (renorm + output projection of the
    # previous query block), pumped between attention pairs so the
    # scalar engine's exp stream never starves at block boundaries.
    # During attention for block j we must fully emit qkv for block j+1
    # (force-drained at the end) and the deferred work of block j-1.
    # head: only the 3 q/k units of tb0 are needed before block 0's
    # scores; tb0's 4 v units join the stream (block 0's PVs are pending
    # until block 1 anyway).
    def unit_stream():
        head = emit_qkv_units(0)
        for _ in range(3):
            next(head)
        yield  # account for the partially-consumed generator
        yield from head
        for tb in range(1, NQB):
            yield from emit_qkv_units(tb)

    units = unit_stream()
    next(units)  # emits tb0's q/k units
    units_left = 4 + 7 * (NQB - 1)
    deferred = []

    def make_deferred(jj, o_ps_j):
        state = {}

        def d_renorm():
            state["att"] = emit_renorm_for(jj, o_ps_j)

        def d_out(tt):
            return lambda: emit_outproj_tt(jj, tt, *state["att"])

        return [d_renorm] + [d_out(tt) for tt in range(4)]

    # pipeline head: qkv(0) complete before attention starts
    for _ in emit_qkv_units(0):
        pass
    for j in range(NQB):
        o_ps = [osp.tile([128, QB], F32, tag="o", name=f"ops{j}_{h}") for h in range(3)]
        # pace: spread this block's weave items across its pair slots
        unit_target = min(units_left, 7)
        nslots = 2 * j + 1
        emitted_u = 0
        emitted_d = 0

        def pump(slot):
            nonlocal emitted_u, emitted_d, units_left
            want_u = (unit_target * (slot + 1) + nslots - 1) // nslots
            while emitted_u < want_u and units_left > 0:
                next(units)
                units_left -= 1
                emitted_u += 1
            want_d = (len_d0 * (slot + 1) + nslots - 1) // nslots
            while emitted_d < want_d and deferred:
                deferred.pop(0)()
                emitted_d += 1

        # in the last block, hold back outproj(6) so it can fill the PE
        # idle time under the final renorm chain in the tail
        len_d0 = min(len(deferred), 1) if j == NQB - 1 else len(deferred)
        for g in range(2 * j):
            emit_pair(o_ps, j, g)
            pump(g)
        emit_diag(o_ps, j)
        pump(nslots - 1)
        deferred.extend(make_deferred(j, o_ps))
    flush_pending()
    # tail order: renorm(7) first (its DVE/gpsimd chain runs while the PE
    # executes the held-back outproj(6) groups), then the projections
    deferred = [deferred[4]] + deferred[0:4] + deferred[5:]
    while deferred:
        deferred.pop(0)()


def _build():
    import concourse.bass as bass  # noqa: F401
    import concourse.tile as tile
    import concourse.mybir as mybir
    from concourse import bacc
    from contextlib import ExitStack

    F32 = mybir.dt.float32
    BF16 = mybir.dt.bfloat16
    nc = bacc.Bacc()
    xT = nc.dram_tensor("xT", [C, T], BF16, kind="ExternalInput").ap()
    wqk = nc.dram_tensor("wqk", [C, 3, 128], BF16, kind="ExternalInput").ap()
    wvp = nc.dram_tensor("wvp", [C, 192], BF16, kind="ExternalInput").ap()
    wp01 = nc.dram_tensor("wp01", [128, C], BF16, kind="ExternalInput").ap()
    wp2 = nc.dram_tensor("wp2", [64, C], BF16, kind="ExternalInput").ap()
    consts = nc.dram_tensor("consts", [128, 256], BF16, kind="ExternalInput").ap()
    out = nc.dram_tensor("out", [T, C], F32, kind="ExternalOutput").ap()

    with tile.TileContext(nc) as tc, ExitStack() as ctx:
        _emit(nc, tile, mybir, tc, ctx, (xT, wqk, wvp, wp01, wp2, consts, out))
    nc.compile()
    return nc


def _consts_np():
    import ml_dtypes

    # tril mask [128,128] (keep score[k, q] iff q >= k), stored twice
    consts = np.zeros((128, 256), np.float32)
    p = np.arange(128)[:, None]
    f = np.arange(128)[None, :]
    tril = (f >= p).astype(np.float32)
    consts[:, 0:128] = tril
    consts[:, 128:256] = tril
    return consts.astype(ml_dtypes.bfloat16)


def _shard_inputs(x, Wqkv, Wproj):
    import ml_dtypes

    BF = ml_dtypes.bfloat16
    consts = _consts_np()
    in_maps = []
    for c in range(NCORES):
        b = c // 4
        hs = [3 * (c % 4) + j for j in range(HPC)]
        wqk = np.zeros((C, 3, 128), np.float32)
        wqk[:, 0, 0:64] = Wqkv[:, (0 * H + hs[0]) * D : (0 * H + hs[0] + 1) * D]
        wqk[:, 0, 64:128] = Wqkv[:, (0 * H + hs[1]) * D : (0 * H + hs[1] + 1) * D]
        wqk[:, 1, 0:64] = Wqkv[:, (1 * H + hs[0]) * D : (1 * H + hs[0] + 1) * D]
        wqk[:, 1, 64:128] = Wqkv[:, (1 * H + hs[1]) * D : (1 * H + hs[1] + 1) * D]
        wqk[:, 2, 0:64] = Wqkv[:, (0 * H + hs[2]) * D : (0 * H + hs[2] + 1) * D]
        wqk[:, 2, 64:128] = Wqkv[:, (1 * H + hs[2]) * D : (1 * H + hs[2] + 1) * D]
        wvp = np.zeros((C, 192), np.float32)
        for j, h in enumerate(hs):
            wvp[:, j * 64 : (j + 1) * 64] = Wqkv[
                :, (2 * H + h) * D : (2 * H + h + 1) * D
            ]
        wp01 = np.concatenate(
            [Wproj[hs[0] * D : (hs[0] + 1) * D, :], Wproj[hs[1] * D : (hs[1] + 1) * D, :]]
        ).astype(np.float32)
        wp2 = Wproj[hs[2] * D : (hs[2] + 1) * D, :].astype(np.float32)
        in_maps.append(
            {
                "xT": np.ascontiguousarray(x[b].T).astype(BF),
                "wqk": wqk.astype(BF),
                "wvp": wvp.astype(BF),
                "wp01": wp01.astype(BF),
                "wp2": wp2.astype(BF),
                "consts": consts,
            }
        )
    return in_maps


def kernel(x, Wqkv, Wproj, bproj):
    from concourse.bass_utils import run_bass_kernel_spmd

    x = np.asarray(x, np.float32)
    Wqkv = np.asarray(Wqkv, np.float32)
    Wproj = np.asarray(Wproj, np.float32)
    bproj = np.asarray(bproj, np.float32)

    if "nc" not in _COMPILED:
        _COMPILED["nc"] = _build()
    nc = _COMPILED["nc"]

    in_maps = _shard_inputs(x, Wqkv, Wproj)
    r = run_bass_kernel_spmd(nc, in_maps, list(range(NCORES)))
    LAST["res"] = r
    res = r.results
    out = np.zeros((B, T, C), np.float32)
    for c in range(NCORES):
        out[c // 4] += res[c]["out"]
    out += bproj[None, None, :]
    return out


# revision 31
# speedup vs baseline: 1.0887x; 1.0887x over previous
"""Causal self-attention (B=2, T=4096, C=768, H=12, D=64) on 8 trn2 cores.

Sharding: (B, H) -> 24 (batch, head) pairs, 3 heads per core.
Core c handles batch b = c // 4 and heads 3*(c%4) .. 3*(c%4)+2.

v5: fine-grained weave + trimmed diagonal. Engines run their queues in
order, so overlap is encoded in emission order: qkv projection units
for block j+1 AND the renorm/output-projection of block j-1 are woven
between the attention pairs of block j, keeping the scalar engine's
exp stream (the per-core floor, ~230us) continuous. Score matmuls are
fully row-paired: h0/h1 across partition halves, h2 against itself
(duplicated q/k in partitions 64-127). The 4 diagonal key-tiles of
each block use trimmed query ranges (512/384/256/128) so ~11% of
score/exp/PV work on masked-out regions is skipped; only the leading
128-query triangle of each tile needs a mask. The output projection
contracts h0+h1 in one K=128 matmul (heads stacked on partitions).

Per-core layouts (all bf16 unless noted):
  xT   [768, 4096]   x[b].T so projections stream tokens in the free dim
  wqk  [768, 3, 128] col groups: [Wq_h0|Wq_h1], [Wk_h0|Wk_h1], [Wq_h2|Wk_h2]
  wvp  [768, 192]    [Wv_h0 Wv_h1 Wv_h2]
  wp01 [128, 768]    Wproj rows for h0 (partitions 0-63) and h1 (64-127)
  wp2  [64, 768]     Wproj rows for h2
  consts [128, 2048] 4 causal masks [128,512]
Scores are computed transposed (ST[k, q]) so the PV matmul contracts k
on the partition dim with V in natural [t, d] layout. Row sums ride an
appended ones-column on V (flash-style, no running max).
"""

import numpy as np

B, T, C, H, D = 2, 4096, 768, 12, 64
HPC = 3          # heads per core
NCORES = 8
QB = 512         # query block (psum bank width in fp32)
KT = 128         # key tile
NKT = T // KT    # 32
NQB = T // QB    # 8
VSTRIDE = 200    # per-k-tile column stride in vbig (3*65 used + 5 pad)

_COMPILED = {}
LAST = {}


def _emit(nc, tile, mybir, tc, ctx, aps):
    F32 = mybir.dt.float32
    BF16 = mybir.dt.bfloat16
    EXP = mybir.ActivationFunctionType.Exp
    xT, wqk, wvp, wp01, wp2, consts, out = aps
    CC = C // 128  # 6 contraction chunks for the projections

    wpool = ctx.enter_context(tc.tile_pool(name="w", bufs=1))
    qkvpool = ctx.enter_context(tc.tile_pool(name="qkv", bufs=1))
    xpool = ctx.enter_context(tc.tile_pool(name="x", bufs=6))
    ptpool = ctx.enter_context(tc.tile_pool(name="pt", bufs=7))
    atpool = ctx.enter_context(tc.tile_pool(name="at", bufs=2))
    opool = ctx.enter_context(tc.tile_pool(name="osb", bufs=3))
    rpool = ctx.enter_context(tc.tile_pool(name="r", bufs=3))
    stp = ctx.enter_context(tc.tile_pool(name="stp", bufs=2, space="PSUM"))
    osp = ctx.enter_context(tc.tile_pool(name="osp", bufs=3, space="PSUM"))
    pjp = ctx.enter_context(tc.tile_pool(name="pjp", bufs=1, space="PSUM"))

    # ---- constants and weights (wqk first: first matmul needs only it + x) ----
    wqk_sb = wpool.tile([128, CC * 3 * 128], BF16)
    nc.sync.dma_start(
        wqk_sb[:].rearrange("p (a g m) -> p a g m", a=CC, g=3),
        wqk.rearrange("(a p) g m -> p a g m", p=128),
    )
    xpre = []
    for tb in range(2):
        for half in range(2):
            xt = xpool.tile([128, 3 * QB], BF16, tag="xt", name=f"xpre{tb}_{half}")
            nc.sync.dma_start(
                xt[:].rearrange("p (a t) -> p a t", a=3),
                xT[
                    384 * half : 384 * (half + 1), tb * QB : (tb + 1) * QB
                ].rearrange("(a p) t -> p a t", p=128),
            )
            xpre.append(xt)
    wvp_sb = wpool.tile([128, CC * 192], BF16)
    nc.sync.dma_start(
        wvp_sb[:].rearrange("p (a n) -> p a n", a=CC),
        wvp.rearrange("(a p) n -> p a n", p=128),
    )
    # tril mask [128,128] stored twice side by side (strided double-mask
    # for the two 128-query triangle blocks of each trimmed diagonal tile)
    masks_sb = wpool.tile([128, 256], BF16)
    nc.sync.dma_start(masks_sb[:], consts[:])
    wp01_sb = wpool.tile([128, C], BF16)
    nc.sync.dma_start(wp01_sb[:], wp01)
    wp2_sb = wpool.tile([64, C], BF16)
    nc.sync.dma_start(wp2_sb[:], wp2)

    # ---- qkv storage ----
    # qkT01: [0:T] = qT (h0 rows 0-63, h1 rows 64-127), [T:2T] = kT
    qkT01 = qkvpool.tile([128, 2 * T], BF16)
    # qk2: h2's qT/kT duplicated in both partition halves so its two
    # key-tile score matmuls of a pair can run row-concurrent.
    qk2 = qkvpool.tile([128, 2 * T], BF16)
    vbig = qkvpool.tile([128, NKT * VSTRIDE], BF16)
    vbig3 = vbig[:].rearrange("p (t c) -> p t c", c=VSTRIDE)
    for h in range(3):
        nc.gpsimd.memset(vbig3[:, :, 65 * h + 64 : 65 * h + 65], 1.0)

    def emit_qkv_units(tb):
        """Generator: yields after each ~1-1.5us PE unit."""
        t0 = tb * QB
        if tb < 2:
            xh = xpre[2 * tb : 2 * tb + 2]
        else:
            xh = []
            for half in range(2):
                xt = xpool.tile([128, 3 * QB], BF16, tag="xt")
                nc.sync.dma_start(
                    xt[:].rearrange("p (a t) -> p a t", a=3),
                    xT[384 * half : 384 * (half + 1), t0 : t0 + QB].rearrange(
                        "(a p) t -> p a t", p=128
                    ),
                )
                xh.append(xt)

        def xchunk(cc):
            return xh[cc // 3][:, (cc % 3) * QB : (cc % 3 + 1) * QB]

        # unit: q01 and k01 projections (one [128,1024] psum tile)
        ps_qk = stp.tile([128, 2 * QB], F32, tag="st")
        for cc in range(CC):
            nc.tensor.matmul(
                ps_qk[:, 0:QB],
                wqk_sb[:, (cc * 3 + 0) * 128 : (cc * 3 + 1) * 128],
                xchunk(cc),
                start=(cc == 0),
                stop=(cc == CC - 1),
            )
        nc.vector.tensor_copy(qkT01[:, t0 : t0 + QB], ps_qk[:, 0:QB])
        yield
        for cc in range(CC):
            nc.tensor.matmul(
                ps_qk[:, QB : 2 * QB],
                wqk_sb[:, (cc * 3 + 1) * 128 : (cc * 3 + 2) * 128],
                xchunk(cc),
                start=(cc == 0),
                stop=(cc == CC - 1),
            )
        nc.vector.tensor_copy(qkT01[:, T + t0 : T + t0 + QB], ps_qk[:, QB : 2 * QB])
        yield
        # unit: h2 q (psum rows 0-63, bank 0) and k (rows 64-127, bank 1)
        # col-paired; distinct 2KB regions so start-zeroing can't collide.
        ps_h2 = stp.tile([128, 2 * QB], F32, tag="st")
        for g2 in range(2):
            for cc in range(CC):
                base = (cc * 3 + 2) * 128 + 64 * g2
                nc.tensor.matmul(
                    ps_h2[64 * g2 : 64 * g2 + 64, g2 * QB : (g2 + 1) * QB],
                    wqk_sb[:, base : base + 64],
                    xchunk(cc),
                    start=(cc == 0),
                    stop=(cc == CC - 1),
                )
        nc.vector.tensor_copy(qk2[0:64, t0 : t0 + QB], ps_h2[0:64, 0:QB])
        nc.vector.tensor_copy(qk2[64:128, t0 : t0 + QB], ps_h2[0:64, 0:QB])
        nc.vector.tensor_copy(qk2[0:64, T + t0 : T + t0 + QB], ps_h2[64:128, QB : 2 * QB])
        nc.vector.tensor_copy(
            qk2[64:128, T + t0 : T + t0 + QB], ps_h2[64:128, QB : 2 * QB]
        )
        yield
        # units: v projection, one per k-tile
        for tt in range(4):
            kt = 4 * tb + tt
            ps = pjp.tile([128, QB], F32, tag="m")
            for cc in range(CC):
                nc.tensor.matmul(
                    ps[:, 0:192],
                    xchunk(cc)[:, tt * 128 : (tt + 1) * 128],
                    wvp_sb[:, cc * 192 : (cc + 1) * 192],
                    start=(cc == 0),
                    stop=(cc == CC - 1),
                )
            dst = vbig3[:, kt, 0:195].rearrange("p (h c) -> p h c", c=65)[:, :, 0:64]
            nc.vector.tensor_copy(dst, ps[:, 0:192].rearrange("p (h d) -> p h d", h=3))
            yield

    # score-matmul helper: one score MM for head h, key-tile kt, query
    # range [q0, q0+w), output at st[:, c0:c0+w]. For h0/h1 the row half
    # is fixed (cross-head pairing); for h2 it alternates per kt (self
    # pairing against the duplicated copy in partitions 64-127).
    def score_mm(st, h, kt, q0, w, c0, half):
        if h < 2:
            nc.tensor.matmul(
                st[:, c0 : c0 + w],
                qkT01[64 * h : 64 * h + 64, T + kt * KT : T + (kt + 1) * KT],
                qkT01[64 * h : 64 * h + 64, q0 : q0 + w],
                start=True,
                stop=True,
                tile_position=(64 * h, 0),
            )
        else:
            nc.tensor.matmul(
                st[:, c0 : c0 + w],
                qk2[64 * half : 64 * half + 64, T + kt * KT : T + (kt + 1) * KT],
                qk2[64 * half : 64 * half + 64, q0 : q0 + w],
                start=True,
                stop=True,
                tile_position=(64 * half, 0),
            )

    def pv_mm(o_ps, h, kt, nkt, pt, c0, w, qoff):
        nc.tensor.matmul(
            o_ps[h][:, qoff : qoff + w],
            vbig3[:, kt, 65 * h : 65 * h + 65],
            pt[:, c0 : c0 + w],
            start=(kt == 0),
            stop=(kt == nkt - 1),
        )

    # PV matmuls run one group (pair / diag half) behind their scores:
    # `pending` holds thunks for the previous group's PVs, flushed between
    # this group's h0/h1 ACTs and the h2 score matmuls. So every PV's pt
    # has been ready for ~3us (no exposed dependency latency), the PE has
    # dense work during the ACT latency, and the exp stream stays fed.
    pending = []

    def flush_pending():
        for fn in pending:
            fn()
        pending.clear()

    # off-diagonal pair: two full-width key tiles, no mask.
    # h0/h1 score matmuls interleaved so the row-group pairs issue
    # concurrently (3 slots per 2 key-tiles instead of 4).
    def emit_pair(o_ps, qb, g):
        t0 = qb * QB
        nkt = 4 * qb + 4
        sts = [
            stp.tile([128, 2 * QB], F32, tag="st", name=f"st{qb}_{g}_{h}")
            for h in range(2)
        ]
        for i in range(2):
            for h in range(2):
                score_mm(sts[h], h, 2 * g + i, t0, QB, i * QB, i)
        pts = []
        for h in range(2):
            pt = ptpool.tile([128, 2 * QB], BF16, tag="pt")
            nc.scalar.activation(pt[:], sts[h][:], EXP, scale=float(D) ** -0.5)
            pts.append(pt)
        flush_pending()
        st2 = stp.tile([128, 2 * QB], F32, tag="st", name=f"st{qb}_{g}_2")
        for i in range(2):
            score_mm(st2, 2, 2 * g + i, t0, QB, i * QB, i)
        pt2 = ptpool.tile([128, 2 * QB], BF16, tag="pt")
        nc.scalar.activation(pt2[:], st2[:], EXP, scale=float(D) ** -0.5)
        pts.append(pt2)

        def make_pv(h, i, pt):
            return lambda: pv_mm(o_ps, h, 2 * g + i, nkt, pt, i * QB, QB, 0)

        for i in range(2):
            for h in range(3):
                pending.append(make_pv(h, i, pts[h]))

    # diagonal block: 4 key tiles with trimmed query ranges.
    # tile A holds kt d0 (w=512 at col 0) + d1 (w=384 at col 512);
    # tile B holds kt d2 (w=256 at col 0) + d3 (w=128 at col 512).
    # Each start=True matmul owns its own 2KB PSUM region. The leading
    # 128 queries of each segment get the tril mask (strided double-mask
    # covers cols [0:128] and [512:640] in one DVE op).
    def emit_diag(o_ps, qb):
        t0 = qb * QB
        nkt = 4 * qb + 4
        segs = ((0, 512, 128, 384), (256, 256, 384, 128))  # (qoff0, w0, qoff1, w1)
        for pi, (qo0, w0, qo1, w1) in enumerate(segs):
            kd0, kd1 = 4 * qb + 2 * pi, 4 * qb + 2 * pi + 1

            def mask_pt(pt):
                ptm = pt[:].rearrange("p (a c) -> p a c", c=128)[:, 0:5:4, :]
                nc.vector.tensor_mul(
                    ptm, ptm, masks_sb[:].rearrange("p (a c) -> p a c", c=128)
                )

            sts = [
                stp.tile([128, 2 * QB], F32, tag="st", name=f"std{qb}_{pi}_{h}")
                for h in range(2)
            ]
            for h in range(2):
                score_mm(sts[h], h, kd0, t0 + qo0, w0, 0, 0)
            for h in range(2):
                score_mm(sts[h], h, kd1, t0 + qo1, w1, QB, 1)
            pts = []
            for h in range(2):
                pt = ptpool.tile([128, 2 * QB], BF16, tag="pt")
                nc.scalar.activation(
                    pt[:, 0 : QB + w1], sts[h][:, 0 : QB + w1], EXP,
                    scale=float(D) ** -0.5,
                )
                mask_pt(pt)
                pts.append(pt)
            flush_pending()
            st2 = stp.tile([128, 2 * QB], F32, tag="st", name=f"std{qb}_{pi}_2")
            score_mm(st2, 2, kd0, t0 + qo0, w0, 0, 0)
            score_mm(st2, 2, kd1, t0 + qo1, w1, QB, 1)
            pt2 = ptpool.tile([128, 2 * QB], BF16, tag="pt")
            nc.scalar.activation(
                pt2[:, 0 : QB + w1], st2[:, 0 : QB + w1], EXP, scale=float(D) ** -0.5
            )
            mask_pt(pt2)
            pts.append(pt2)

            def make_pv(h, kd, pt, c0, w, qo):
                return lambda: pv_mm(o_ps, h, kd, nkt, pt, c0, w, qo)

            for h in range(3):
                pending.append(make_pv(h, kd0, pts[h], 0, w0, qo0))
            for h in range(3):
                pending.append(make_pv(h, kd1, pts[h], QB, w1, qo1))

    def emit_renorm_head(qb, o_ps, h, att01, att2):
        au = atpool.tile([64, QB], F32, tag=f"au{h}", name=f"au{qb}_{h}")
        nc.vector.tensor_copy(au[:], o_ps[h][0:64, :])
        srow = rpool.tile([1, QB], F32, tag="sr")
        nc.vector.tensor_copy(srow[:], o_ps[h][64:65, :])
        rs = rpool.tile([1, QB], F32, tag="r")
        nc.vector.reciprocal_approx_fast(rs[:], srow[:])
        rbc = atpool.tile([64, QB], F32, tag=f"rbc{h}", name=f"rbc{qb}_{h}")
        nc.gpsimd.partition_broadcast(rbc[:], rs[:])
        dstmul = (att01[0:64, :], att01[64:128, :], att2[:])[h]
        nc.vector.tensor_mul(dstmul, au[:], rbc[:])

    def alloc_att(qb):
        att01 = atpool.tile([128, QB], BF16, tag="att01", name=f"att01_{qb}")
        att2 = atpool.tile([64, QB], BF16, tag="att2", name=f"att2_{qb}")
        return (att01, att2)

    def emit_renorm_for(qb, o_ps):
        att01, att2 = alloc_att(qb)
        for h in range(3):
            emit_renorm_head(qb, o_ps, h, att01, att2)
        return (att01, att2)

    def emit_outproj_tt(qb, tt, att01, att2, pool=None, pshape=None):
        t0 = qb * QB
        osb = opool.tile([128, C], F32, tag="osb")
        for j, (c0, cw) in enumerate(((0, 512), (512, 256))):
            pps = (pool or pjp).tile(pshape or [128, QB], F32, tag="m" if pool is None else "st")
            nc.tensor.matmul(
                pps[:, 0:cw],
                att01[:, tt * 128 : (tt + 1) * 128],
                wp01_sb[:, c0 : c0 + cw],
                start=True,
                stop=False,
            )
            nc.tensor.matmul(
                pps[:, 0:cw],
                att2[:, tt * 128 : (tt + 1) * 128],
                wp2_sb[:, c0 : c0 + cw],
                start=False,
                stop=True,
            )
            nc.vector.tensor_copy(osb[:, c0 : c0 + cw], pps[:, 0:cw])
        nc.sync.dma_start(out[t0 + tt * 128 : t0 + (tt + 1) * 128, :], osb[:])

    # weave: a single stream of qkv units (all token blocks in sequence)
    # plus a deferred-work queue (renorm + output projection of the
    # previous query block), pumped between attention pairs so the
    # scalar engine's exp stream never starves at block boundaries.
    # During attention for block j we must fully emit qkv for block j+1
    # (force-drained at the end) and the deferred work of block j-1.
    def unit_stream():
        for tb in range(1, NQB):
            yield from emit_qkv_units(tb)

    units = unit_stream()
    units_left = 7 * (NQB - 1)
    deferred = []

    def make_deferred(jj, o_ps_j):
        state = {}

        def d_renorm():
            state["att"] = emit_renorm_for(jj, o_ps_j)

        def d_out(tt):
            return lambda: emit_outproj_tt(jj, tt, *state["att"])

        return [d_renorm] + [d_out(tt) for tt in range(4)]

    # pipeline head: qkv(0) complete before attention starts
    for _ in emit_qkv_units(0):
        pass
    for j in range(NQB):
        o_ps = [osp.tile([65, QB], F32, tag="o", name=f"ops{j}_{h}") for h in range(3)]
        # pace: spread this block's weave items across its pair slots
        unit_target = min(units_left, 7)
        nslots = 2 * j + 1
        emitted_u = 0
        emitted_d = 0

        def pump(slot):
            nonlocal emitted_u, emitted_d, units_left
            want_u = (unit_target * (slot + 1) + nslots - 1) // nslots
            while emitted_u < want_u and units_left > 0:
                next(units)
                units_left -= 1
                emitted_u += 1
            want_d = (len_d0 * (slot + 1) + nslots - 1) // nslots
            while emitted_d < want_d and deferred:
                deferred.pop(0)()
                emitted_d += 1

        len_d0 = len(deferred)
        for g in range(2 * j):
            emit_pair(o_ps, j, g)
            pump(g)
        emit_diag(o_ps, j)
        pump(nslots - 1)
        if j < NQB - 1:
            deferred.extend(make_deferred(j, o_ps))
    # tail: the last diag group's pending PVs run per head, each head's
    # renorm chain starting as soon as its own PVs are done (the chain
    # overlaps the other heads' PVs on the PE). The final block's output
    # projection goes through double-buffered stp psum tiles (the score
    # pipeline is finished) so its copies overlap the matmuls.
    att01, att2 = alloc_att(NQB - 1)
    for h in range(3):
        pending[h]()
        pending[h + 3]()
        emit_renorm_head(NQB - 1, o_ps, h, att01, att2)
    pending.clear()
    for tt in range(4):
        emit_outproj_tt(NQB - 1, tt, att01, att2, pool=stp, pshape=[128, 2 * QB])
    while deferred:
        deferred.pop(0)()


def _build():
    import concourse.bass as bass  # noqa: F401
    import concourse.tile as tile
    import concourse.mybir as mybir
    from concourse import bacc
    from contextlib import ExitStack

    F32 = mybir.dt.float32
    BF16 = mybir.dt.bfloat16
    nc = bacc.Bacc()
    xT = nc.dram_tensor("xT", [C, T], BF16, kind="ExternalInput").ap()
    wqk = nc.dram_tensor("wqk", [C, 3, 128], BF16, kind="ExternalInput").ap()
    wvp = nc.dram_tensor("wvp", [C, 192], BF16, kind="ExternalInput").ap()
    wp01 = nc.dram_tensor("wp01", [128, C], BF16, kind="ExternalInput").ap()
    wp2 = nc.dram_tensor("wp2", [64, C], BF16, kind="ExternalInput").ap()
    consts = nc.dram_tensor("consts", [128, 256], BF16, kind="ExternalInput").ap()
    out = nc.dram_tensor("out", [T, C], F32, kind="ExternalOutput").ap()

    with tile.TileContext(nc) as tc, ExitStack() as ctx:
        _emit(nc, tile, mybir, tc, ctx, (xT, wqk, wvp, wp01, wp2, consts, out))
    nc.compile()
    return nc


def _consts_np():
    import ml_dtypes

    # tril mask [128,128] (keep score[k, q] iff q >= k), stored twice
    consts = np.zeros((128, 256), np.float32)
    p = np.arange(128)[:, None]
    f = np.arange(128)[None, :]
    tril = (f >= p).astype(np.float32)
    consts[:, 0:128] = tril
    consts[:, 128:256] = tril
    return consts.astype(ml_dtypes.bfloat16)


def _shard_inputs(x, Wqkv, Wproj):
    import ml_dtypes

    BF = ml_dtypes.bfloat16
    consts = _consts_np()
    in_maps = []
    for c in range(NCORES):
        b = c // 4
        hs = [3 * (c % 4) + j for j in range(HPC)]
        wqk = np.zeros((C, 3, 128), np.float32)
        wqk[:, 0, 0:64] = Wqkv[:, (0 * H + hs[0]) * D : (0 * H + hs[0] + 1) * D]
        wqk[:, 0, 64:128] = Wqkv[:, (0 * H + hs[1]) * D : (0 * H + hs[1] + 1) * D]
        wqk[:, 1, 0:64] = Wqkv[:, (1 * H + hs[0]) * D : (1 * H + hs[0] + 1) * D]
        wqk[:, 1, 64:128] = Wqkv[:, (1 * H + hs[1]) * D : (1 * H + hs[1] + 1) * D]
        wqk[:, 2, 0:64] = Wqkv[:, (0 * H + hs[2]) * D : (0 * H + hs[2] + 1) * D]
        wqk[:, 2, 64:128] = Wqkv[:, (1 * H + hs[2]) * D : (1 * H + hs[2] + 1) * D]
        wvp = np.zeros((C, 192), np.float32)
        for j, h in enumerate(hs):
            wvp[:, j * 64 : (j + 1) * 64] = Wqkv[
                :, (2 * H + h) * D : (2 * H + h + 1) * D
            ]
        wp01 = np.concatenate(
            [Wproj[hs[0] * D : (hs[0] + 1) * D, :], Wproj[hs[1] * D : (hs[1] + 1) * D, :]]
        ).astype(np.float32)
        wp2 = Wproj[hs[2] * D : (hs[2] + 1) * D, :].astype(np.float32)
        in_maps.append(
            {
                "xT": np.ascontiguousarray(x[b].T).astype(BF),
                "wqk": wqk.astype(BF),
                "wvp": wvp.astype(BF),
                "wp01": wp01.astype(BF),
                "wp2": wp2.astype(BF),
                "consts": consts,
            }
        )
    return in_maps


def kernel(x, Wqkv, Wproj, bproj):
    from concourse.bass_utils import run_bass_kernel_spmd

    x = np.asarray(x, np.float32)
    Wqkv = np.asarray(Wqkv, np.float32)
    Wproj = np.asarray(Wproj, np.float32)
    bproj = np.asarray(bproj, np.float32)

    if "nc" not in _COMPILED:
        _COMPILED["nc"] = _build()
    nc = _COMPILED["nc"]

    in_maps = _shard_inputs(x, Wqkv, Wproj)
    r = run_bass_kernel_spmd(nc, in_maps, list(range(NCORES)))
    LAST["res"] = r
    res = r.results
    out = np.zeros((B, T, C), np.float32)
    for c in range(NCORES):
        out[c // 4] += res[c]["out"]
    out += bproj[None, None, :]
    return out
